# revision 20
# baseline (speedup 1.0000x reference)
"""CompressedLinear on 8 Trainium2 NeuronCores.

out[b,s,o] = sum_i x[b,s,i] * (w_int8[o,i] * scale[o]) + bias[o]
  x: [4, 2048, 4096] f32, w_int8: [16384, 4096] int32 (codes in [-64,63]),
  scale/bias: [16384] f32 -> out: [4, 2048, 16384] f32

Strategy (tensor-parallel over out_features):
  - Each of the 8 cores owns a 2048-row slice of W/scale/bias and computes
    out[:, :, c*2048:(c+1)*2048]; x is replicated.
  - Weights codes are exact in bf16; scale is applied AFTER the matmul
    (per-out-feature), so the matmul itself is integer-exact in bf16.
  - x (f32) is split host-side into x_hi + x_lo (two bf16 arrays) and both
    are matmul'd against the bf16 codes, accumulating in f32 PSUM -> ~1e-5
    relative error at 2x bf16 matmul cost (vs 4x for native f32 matmul).
  - Per core loop: stationary operand = 128-token column block of x^T,
    moving operand = w^T; PSUM holds [128 tokens, 4x512 outfeat]; 32 k-tiles
    x {hi,lo} x 4 banks = 256 matmuls per token tile, then one fused
    scale-mult + bias-add epilogue on DVE and a DMA store.

All data layout transforms (transpose, hi/lo split, int8->bf16 cast,
scale/bias broadcast) are host-side numpy; gather is a concat.
"""

import os

import numpy as np
import ml_dtypes

BF16 = ml_dtypes.bfloat16

OUT, IN = 16384, 4096
B, S = 4, 2048
TOK = B * S            # 8192 tokens
NCORES = 8
OSH = OUT // NCORES    # 2048 out-features per core
KT = IN // 128         # 32 k-tiles
TT = TOK // 128        # 64 token tiles
NB = OSH // 512        # 4 psum banks per token tile

_last_results = None   # BassKernelResults of the most recent run (for test.py)


def _build_program():
    from contextlib import ExitStack

    import concourse.bass as bass
    import concourse.tile as tile
    from concourse import mybir

    f32 = mybir.dt.float32
    bf16 = mybir.dt.bfloat16

    nc = bass.Bass()
    xhi_d = nc.declare_dram_parameter("xhi", [TT, 128, KT, 128], bf16, isOutput=False)
    xlo_d = nc.declare_dram_parameter("xlo", [TT, 128, KT, 128], bf16, isOutput=False)
    w_d = nc.declare_dram_parameter("w", [128, KT, OSH], bf16, isOutput=False)
    scale_d = nc.declare_dram_parameter("scale", [128, NB, 512], f32, isOutput=False)
    bias_d = nc.declare_dram_parameter("bias", [128, NB, 512], f32, isOutput=False)
    out_d = nc.declare_dram_parameter("out", [TT, 128, NB, 512], f32, isOutput=True)

    from concourse.tile import add_dep_helper

    with tile.TileContext(nc) as tc, ExitStack() as ctx:
        wpool = ctx.enter_context(tc.tile_pool(name="w", bufs=1))
        cpool = ctx.enter_context(tc.tile_pool(name="consts", bufs=1))
        xpool = ctx.enter_context(tc.tile_pool(name="x", bufs=2))
        opool = ctx.enter_context(tc.tile_pool(name="o", bufs=2))
        pspool = ctx.enter_context(tc.tile_pool(name="ps", bufs=2, space="PSUM"))

        w_sb = wpool.tile([128, KT, OSH], bf16)
        w_dma = nc.sync.dma_start(w_sb[:], w_d[:])
        scale_sb = cpool.tile([128, NB, 512], f32, tag="scale")
        scale_dma = nc.sync.dma_start(scale_sb[:], scale_d[:])
        bias_sb = cpool.tile([128, NB, 512], f32, tag="bias")
        bias_dma = nc.sync.dma_start(bias_sb[:], bias_d[:])

        # Per-iteration disjoint scratch columns -> the carrier ops carry no
        # WAW deps of their own.
        scratch = cpool.tile([1, TT], f32, tag="scratch")
        dummy = cpool.tile([1, 3 * TT], f32, tag="dummy")
        dveA = cpool.tile([1, TT], f32, tag="dveA")
        dveB = cpool.tile([1, TT], f32, tag="dveB")
        # Preamble DVE carriers: observe the scale/bias const loads on DVE so
        # no steady-state DVE op pairs a DMAHW wait with another wait.
        pre = cpool.tile([1, 2], f32, tag="pre")
        nc.vector.tensor_copy(pre[:, 0:1], scale_sb[:1, 0, :1])
        nc.vector.tensor_copy(pre[:, 1:2], bias_sb[:1, 0, :1])

        psum_readers = []  # the scale-mult (last psum reader) per iteration
        last_mms = []  # final matmul per iteration
        out_dmas = []
        out_copies = []
        x_dmas = []
        adds = []

        # Hardware sync-wait slots are tiny (1 per PE LW/MM and per SWDGE
        # DMA, 2 per HWDGE DMA), and Tile's wait assignment is per-proc
        # minimal but not transitive. So every cross-engine dependency is
        # absorbed by a dedicated cheap "carrier" op on the consuming engine,
        # with explicit ordering edges so the scheduler keeps each carrier
        # ahead of its dependents and every instruction introduces at most
        # one new wait.
        def order(after, before):
            add_dep_helper(after.ins, before.ins, sync=False, reason="carrier order")

        for t in range(TT):
            xhi = xpool.tile([128, KT, 128], bf16, tag="xhi")
            xlo = xpool.tile([128, KT, 128], bf16, tag="xlo")
            # POOL carrier chain, one wait each: gen-2 x-load DMAs (their
            # lane sems would otherwise ride the new DMAs as WAW waits) and
            # gen-2 matmuls (x slot readers), before the x-slot rewrite.
            ms1 = nc.gpsimd.memset(dummy[:, 3 * t : 3 * t + 1], 0)
            ms2 = nc.gpsimd.memset(dummy[:, 3 * t + 1 : 3 * t + 2], 0)
            ms3 = nc.gpsimd.memset(dummy[:, 3 * t + 2 : 3 * t + 3], 0)
            order(ms2, ms1)
            order(ms3, ms2)
            if t >= 2:
                add_dep_helper(
                    ms1.ins, x_dmas[t - 2][0].ins, reason="x WAW lane via carrier"
                )
                add_dep_helper(
                    ms2.ins, x_dmas[t - 2][1].ins, reason="x WAW lane via carrier"
                )
                add_dep_helper(
                    ms3.ins,
                    last_mms[t - 2].ins,
                    reason="x slot reuse gated on POOL carrier",
                )
            d1 = nc.gpsimd.dma_start(xlo[:], xlo_d[t])
            d2 = nc.gpsimd.dma_start(xhi[:], xhi_d[t])
            order(d1, ms3)
            order(d2, ms3)
            x_dmas.append((d1, d2))

            ps = pspool.tile([128, NB, 512], f32)
            # PE carrier: guard LDWEIGHTS absorbing the psum-slot-free (DVE)
            # wait so the first real matmul only waits on PE.
            guard = nc.tensor.ldweights(w_sb[:, 0, :128])
            if t >= 2:
                add_dep_helper(
                    guard.ins,
                    psum_readers[t - 2].ins,
                    reason="psum slot reuse gated on guard ldweights",
                )
            first_mm = None
            for k in range(KT):
                for xt, first, last in (
                    (xhi, k == 0, False),
                    (xlo, False, k == KT - 1),
                ):
                    for j in range(NB):
                        mm = nc.tensor.matmul(
                            ps[:, j, :],
                            xt[:, k, :],
                            w_sb[:, k, j * 512 : (j + 1) * 512],
                            start=first,
                            stop=last,
                        )
                        if first_mm is None:
                            first_mm = mm
            order(first_mm, guard)
            last_mms.append(mm)

            ob = opool.tile([128, NB, 512], f32)
            # DVE carriers: absorb the ob-slot WAR deps (gen-2 out-store DMA
            # and gen-2 POOL scratch copy) ahead of the scale-mult.
            c1 = nc.vector.tensor_copy(dveA[:, t : t + 1], scale_sb[:1, 0, :1])
            c2 = nc.vector.tensor_copy(dveB[:, t : t + 1], scale_sb[:1, 0, :1])
            if t >= 2:
                add_dep_helper(
                    c1.ins, out_dmas[t - 2].ins, reason="ob reuse vs out dma"
                )
                add_dep_helper(
                    c2.ins, out_copies[t - 2].ins, reason="ob reuse vs pool copy"
                )
            mult = nc.vector.tensor_tensor(
                ob[:], ps[:], scale_sb[:], mybir.AluOpType.mult
            )
            order(mult, c1)
            order(mult, c2)
            psum_readers.append(mult)
            adds.append(
                nc.vector.tensor_tensor(ob[:], ob[:], bias_sb[:], mybir.AluOpType.add)
            )
            # POOL carrier: RAW on ob -> absorbs the DVE wait ahead of the
            # out-store.
            cp = nc.gpsimd.tensor_copy(scratch[:, t : t + 1], ob[:1, 0, :1])
            od = nc.gpsimd.dma_start(out_d[t], ob[:])
            order(od, cp)
            out_copies.append(cp)
            out_dmas.append(od)

        # Tail carriers: SP nops, one wait each, observing every outstanding
        # sem (PE, DVE, Pool, all SWDGE lanes, preamble HWDGE lanes) so the
        # kernel-tail SP drain doesn't exceed its sync-wait slots.
        tail_deps = [
            last_mms[-1],
            adds[-1],
            out_copies[-1],
            w_dma,
            scale_dma,
            bias_dma,
        ]
        for i in (1, 2, 3):
            tail_deps += [out_dmas[-i], x_dmas[-i][1], x_dmas[-i][0]]
        for i, dep in enumerate(tail_deps):
            nop = nc.engines[mybir.EngineType.SP].nop(
                nofuse=True, hint=f"tail_carrier_{i}"
            )
            add_dep_helper(nop.ins, dep.ins, reason="tail drain carrier")

    return nc


def kernel(x, weight_int8, scale, bias):
    global _last_results
    from concourse.bass_utils import run_bass_kernel_spmd

    x = np.asarray(x)
    weight_int8 = np.asarray(weight_int8)
    scale = np.asarray(scale, dtype=np.float32)
    bias = np.asarray(bias, dtype=np.float32)

    # x^T [IN, TOK], hi/lo bf16 split, tiled to [TT, 128p(IN), KT, 128(tok)]
    xT = np.ascontiguousarray(x.reshape(TOK, IN).astype(np.float32).T)
    x_hi = xT.astype(BF16)
    x_lo = (xT - x_hi.astype(np.float32)).astype(BF16)
    x_hi = np.ascontiguousarray(
        x_hi.reshape(KT, 128, TT, 128).transpose(2, 1, 0, 3)
    )
    x_lo = np.ascontiguousarray(
        x_lo.reshape(KT, 128, TT, 128).transpose(2, 1, 0, 3)
    )

    in_maps = []
    for c in range(NCORES):
        wc = weight_int8[c * OSH : (c + 1) * OSH].astype(np.float32).astype(BF16)
        # w^T [IN, OSH] tiled to [128p(IN), KT, OSH]
        wp = np.ascontiguousarray(wc.T.reshape(KT, 128, OSH).transpose(1, 0, 2))
        sc = np.ascontiguousarray(
            np.broadcast_to(scale[c * OSH : (c + 1) * OSH], (128, OSH))
        ).reshape(128, NB, 512)
        bc = np.ascontiguousarray(
            np.broadcast_to(bias[c * OSH : (c + 1) * OSH], (128, OSH))
        ).reshape(128, NB, 512)
        in_maps.append({"xhi": x_hi, "xlo": x_lo, "w": wp, "scale": sc, "bias": bc})

    nc = _build_program()
    trace = bool(os.environ.get("KERNEL_TRACE"))
    kwargs = {}
    if trace:
        # Local-only profiling: stub the bucket upload and install the axon
        # NTFF hook (the image's antenv stub lacks axon_hooks).
        import sys
        import types

        from concourse import bass_utils as _bu

        _bu.upload_artifacts = lambda tmpdir: "local://" + tmpdir
        if "antenv.axon_hooks" not in sys.modules:
            import antenv

            mod = types.ModuleType("antenv.axon_hooks")
            _holder = [None]
            mod.set_axon_ntff_profile_hook = lambda h: _holder.__setitem__(0, h)
            mod.get_axon_ntff_profile_hook = lambda: _holder[0]
            antenv.axon_hooks = mod
            sys.modules["antenv.axon_hooks"] = mod
        from antenv.axon_hooks import (
            get_axon_ntff_profile_hook,
            set_axon_ntff_profile_hook,
        )

        if get_axon_ntff_profile_hook() is None:
            from trn_agent_boot.trn_boot import _ntff_profile_via_ctypes

            set_axon_ntff_profile_hook(
                _ntff_profile_via_ctypes(
                    os.environ.get("PJRT_LIBRARY_PATH", "/opt/axon/libaxon_pjrt.so")
                )
            )
        tmpdir = os.environ.get("KERNEL_TRACE_DIR")
        if tmpdir:
            os.makedirs(tmpdir, exist_ok=True)
            kwargs["tmpdir"] = tmpdir

    res = run_bass_kernel_spmd(
        nc,
        in_maps,
        list(range(NCORES)),
        trace=trace,
        **kwargs,
    )
    _last_results = res

    parts = [res.results[c]["out"].reshape(TOK, OSH) for c in range(NCORES)]
    return np.concatenate(parts, axis=1).reshape(B, S, OUT)
